# revision 3
# baseline (speedup 1.0000x reference)
"""Trainium2 Bass kernel for nn_GaussianEmbedding.

Y[b,n,c] = h[0,c] + sum_{i=1..8} h[i,c] * diag(A^i)[b,n]

Math: with full powers A^1..A^4 available, every diag(A^i) for i=2..8 is a
"rows of A^p dot cols of A^q" (p+q=i) — computed on the tensor engine as 16
diagonal [128,128] blocks, 1/16 the cost of a full matmul.  So only 3 full
2048^3 matmuls (A2 = A@A, A3 = A2@A as A3^T = A^T@A2^T, A4 = A3@A) in bf16
instead of the reference's 7 in fp32.

Sharding: data-parallel over batch — one [2048,2048] A per NeuronCore, 8 cores.

Per-core schedule (two 8.4MB bf16 SBUF residents max, rest streamed):
  P1: A2   = A@A     (lhsT = A^T chunks via DMA-transpose of A; rhs = A)  + D1, D2
  P2: A3^T = A^T@A2^T(lhsT = A col-slices; rhs = A2^T via DMA-transpose)  + D3
  P3: A4   = A3@A    (lhsT = A3^T slices; rhs = A)                        + D4
  P4: D5..D8         (lhsT = {A,A2,A4}^T DMA-transpose chunks + A3^T; rhs = A4)
  P5: Y = D'^T @ hh  (PE transpose of diag tile + K=9 fp32 matmul)
"""

import numpy as np

NUM_TERMS = 8
C = 64
BATCH = 8
N = 2048
P = 128
N_CORES = 8

_RUNNER = None
_NC = None


def _bf16(x: np.ndarray):
    """Fast float32 -> bfloat16 with round-to-nearest-even (vectorized)."""
    import ml_dtypes

    u = np.ascontiguousarray(x, np.float32).view(np.uint32)
    r = (u >> np.uint32(16)) & np.uint32(1)
    out = ((u + np.uint32(0x7FFF) + r) >> np.uint32(16)).astype(np.uint16)
    return out.view(ml_dtypes.bfloat16)


def _build(n: int = N):
    import concourse.tile as tile
    from concourse import bacc, mybir
    from concourse.masks import make_identity

    f32 = mybir.dt.float32
    bf16 = mybir.dt.bfloat16
    mult = mybir.AluOpType.mult
    add = mybir.AluOpType.add
    AX = mybir.AxisListType.X

    ko = n // P      # contraction chunks / row-slabs
    nb = n // P      # diagonal blocks
    nw = n // 512    # 512-wide output chunks per row-block

    nc = bacc.Bacc("TRN2", target_bir_lowering=False, num_devices=N_CORES)

    A_in = nc.declare_dram_parameter("A", [n, n], bf16, isOutput=False)
    hh_in = nc.declare_dram_parameter("hh", [16, C], f32, isOutput=False)
    Y_out = nc.declare_dram_parameter("Y", [n, C], f32, isOutput=True)

    A2_d = nc.dram_tensor("A2_d", [n, n], bf16)
    A3T_d = nc.dram_tensor("A3T_d", [n, n], bf16)
    A4_d = nc.dram_tensor("A4_d", [n, n], bf16)

    with tile.TileContext(nc) as tc:
        with (
            tc.tile_pool(name="big", bufs=1) as big,
            tc.tile_pool(name="small", bufs=1) as small,
            tc.tile_pool(name="chunks", bufs=3) as chunks,
            tc.tile_pool(name="ev", bufs=4) as evp,
            tc.tile_pool(name="mmps", bufs=2, space="PSUM") as psp,
            tc.tile_pool(name="dgps", bufs=2, space="PSUM") as dpsp,
        ):
            # Two big SBUF residents, manually phase-multiplexed:
            #   bigA: A (P1-P3), then A4 (P4).   bigB: A2^T (P2), then A3^T (P3-P4).
            bigA = big.tile([P, ko, n], bf16, tag="bigA")
            bigB = big.tile([P, ko, n], bf16, tag="bigB")

            identf = small.tile([P, P], f32)
            identb = small.tile([P, P], bf16)
            make_identity(nc, identf)
            make_identity(nc, identb)
            hh_sb = small.tile([16, C], f32)
            nc.sync.dma_start(hh_sb, hh_in[:, :])
            # Dall[:, blk, t]: t=0..7 -> diag(A^(t+1)) for nodes blk*128+part; t=8 -> 1.0
            Dall = small.tile([P, nb, 16], f32)
            nc.any.memset(Dall[:, :, 8:9], 1.0)

            def diag_block(tslot, m, lhsT_chunk, rhs_sb):
                """Dall[:, m, tslot] = diag-block m of (lhsT_chunk^T @ rhs_sb block m)."""
                dps = dpsp.tile([P, P], f32, tag="dg")
                for kk in range(ko):
                    nc.tensor.matmul(
                        dps,
                        lhsT_chunk[:, kk, :],
                        rhs_sb[:, kk, m * P : (m + 1) * P],
                        start=(kk == 0),
                        stop=(kk == ko - 1),
                    )
                dt = evp.tile([P, P], f32, tag="dtmp")
                nc.vector.tensor_tensor(dt, dps, identf, mult)
                nc.vector.tensor_reduce(Dall[:, m, tslot : tslot + 1], dt, AX, add)

            # ---- P0: load A into bigA ([part=row%128, row//128, col]) ----
            for mb in range(ko):
                nc.sync.dma_start(bigA[:, mb, :], A_in[mb * P : (mb + 1) * P, :])

            # ---- P1: A2 = A@A ; D1 = diag(A); D2 = diag(A·A) ----
            for m in range(nb):
                atm = chunks.tile([P, ko, P], bf16, tag="atm")
                nc.sync.dma_start_transpose(atm, A_in[m * P : (m + 1) * P, :])
                for nci in range(nw):
                    ps = psp.tile([P, 512], f32, tag="mm")
                    for kk in range(ko):
                        nc.tensor.matmul(
                            ps,
                            atm[:, kk, :],
                            bigA[:, kk, nci * 512 : (nci + 1) * 512],
                            start=(kk == 0),
                            stop=(kk == ko - 1),
                        )
                    ev = evp.tile([P, 512], bf16, tag="ev")
                    nc.any.tensor_copy(out=ev, in_=ps)
                    nc.sync.dma_start(
                        A2_d[m * P : (m + 1) * P, nci * 512 : (nci + 1) * 512], ev
                    )
                diag_block(1, m, atm, bigA)  # D2
                dt1 = evp.tile([P, P], f32, tag="dtmp")
                nc.vector.tensor_tensor(
                    dt1, bigA[:, m, m * P : (m + 1) * P], identb, mult
                )
                nc.vector.tensor_reduce(Dall[:, m, 0:1], dt1, AX, add)  # D1

            # ---- P2: A3^T = A^T @ A2^T ; D3 = diag(A2·A) ----
            for mb in range(ko):
                nc.sync.dma_start_transpose(
                    bigB[:, :, mb * P : (mb + 1) * P], A2_d[mb * P : (mb + 1) * P, :]
                )
            for m in range(nb):
                lh = bigA[:, :, m * P : (m + 1) * P]  # A col-slice: lhsT chunks
                for nci in range(nw):
                    ps = psp.tile([P, 512], f32, tag="mm")
                    for kk in range(ko):
                        nc.tensor.matmul(
                            ps,
                            lh[:, kk, :],
                            bigB[:, kk, nci * 512 : (nci + 1) * 512],
                            start=(kk == 0),
                            stop=(kk == ko - 1),
                        )
                    ev = evp.tile([P, 512], bf16, tag="ev")
                    nc.any.tensor_copy(out=ev, in_=ps)
                    nc.sync.dma_start(
                        A3T_d[m * P : (m + 1) * P, nci * 512 : (nci + 1) * 512], ev
                    )
                diag_block(2, m, bigB[:, :, m * P : (m + 1) * P], bigA)  # D3

            # ---- P3: A4 = A3@A (lhsT = A3^T slices) ; D4 = diag(A3·A) ----
            for mb in range(ko):
                nc.sync.dma_start(bigB[:, mb, :], A3T_d[mb * P : (mb + 1) * P, :])
            for m in range(nb):
                lh = bigB[:, :, m * P : (m + 1) * P]
                for nci in range(nw):
                    ps = psp.tile([P, 512], f32, tag="mm")
                    for kk in range(ko):
                        nc.tensor.matmul(
                            ps,
                            lh[:, kk, :],
                            bigA[:, kk, nci * 512 : (nci + 1) * 512],
                            start=(kk == 0),
                            stop=(kk == ko - 1),
                        )
                    ev = evp.tile([P, 512], bf16, tag="ev")
                    nc.any.tensor_copy(out=ev, in_=ps)
                    nc.sync.dma_start(
                        A4_d[m * P : (m + 1) * P, nci * 512 : (nci + 1) * 512], ev
                    )
                diag_block(3, m, lh, bigA)  # D4

            # ---- P4: A4 -> bigA ; D5..D8 against rhs=A4 ----
            for mb in range(ko):
                nc.sync.dma_start(bigA[:, mb, :], A4_d[mb * P : (mb + 1) * P, :])
            for b in range(nb):
                atb = chunks.tile([P, ko, P], bf16, tag="atb", bufs=2)
                nc.sync.dma_start_transpose(atb, A_in[b * P : (b + 1) * P, :])
                a2tb = chunks.tile([P, ko, P], bf16, tag="a2tb", bufs=2)
                nc.sync.dma_start_transpose(a2tb, A2_d[b * P : (b + 1) * P, :])
                a4tb = chunks.tile([P, ko, P], bf16, tag="a4tb", bufs=2)
                nc.sync.dma_start_transpose(a4tb, A4_d[b * P : (b + 1) * P, :])
                diag_block(4, b, atb, bigA)  # D5 = diag(A·A4)
                diag_block(5, b, a2tb, bigA)  # D6 = diag(A2·A4)
                diag_block(6, b, bigB[:, :, b * P : (b + 1) * P], bigA)  # D7
                diag_block(7, b, a4tb, bigA)  # D8 = diag(A4·A4)

            # ---- P5: Y[n,c] = sum_t Dall[n,t] * hh[t,c] ----
            DT = small.tile([16, nb, P], f32)
            for no in range(nb):
                tp = dpsp.tile([16, P], f32, tag="tp")
                nc.tensor.transpose(tp[:9, :], Dall[:, no, 0:9], identf)
                nc.any.tensor_copy(out=DT[:9, no, :], in_=tp[:9, :])
            y_sb = small.tile([P, nb, C], f32)
            for no in range(nb):
                yp = dpsp.tile([P, C], f32, tag="yp")
                nc.tensor.matmul(
                    yp, DT[:9, no, :], hh_sb[:9, :], start=True, stop=True
                )
                nc.any.tensor_copy(out=y_sb[:, no, :], in_=yp)
            nc.sync.dma_start(
                Y_out.ap().rearrange("(no ni) c -> ni no c", ni=P), y_sb
            )

    nc.compile()
    return nc


def _make_runner(nc):
    """Cached jitted SPMD executor (mirrors bass2jax.run_bass_via_pjrt)."""
    import jax
    from jax.experimental.shard_map import shard_map
    from jax.sharding import Mesh, PartitionSpec

    import concourse.mybir as mybir
    from concourse.bass2jax import (
        _bass_exec_p,
        install_neuronx_cc_hook,
        partition_id_tensor,
    )

    install_neuronx_cc_hook()
    partition_name = nc.partition_id_tensor.name if nc.partition_id_tensor else None

    in_names, out_names, out_avals, zero_outs = [], [], [], []
    for alloc in nc.m.functions[0].allocations:
        if not isinstance(alloc, mybir.MemoryLocationSet):
            continue
        name = alloc.memorylocations[0].name
        if alloc.kind == "ExternalInput":
            if name != partition_name:
                in_names.append(name)
        elif alloc.kind == "ExternalOutput":
            shape = tuple(alloc.tensor_shape)
            dtype = mybir.dt.np(alloc.dtype)
            out_names.append(name)
            out_avals.append(jax.core.ShapedArray(shape, dtype))
            zero_outs.append(np.zeros(shape, dtype))
    n_params = len(in_names)
    n_outs = len(out_avals)
    all_in_names = list(in_names) + list(out_names)
    if partition_name is not None:
        all_in_names.append(partition_name)

    def _body(*args):
        operands = list(args)
        if partition_name is not None:
            operands.append(partition_id_tensor())
        outs = _bass_exec_p.bind(
            *operands,
            out_avals=tuple(out_avals),
            in_names=tuple(all_in_names),
            out_names=tuple(out_names),
            lowering_input_output_aliases=(),
            sim_require_finite=True,
            sim_require_nnan=True,
            nc=nc,
        )
        return tuple(outs)

    devices = jax.devices()[:N_CORES]
    assert len(devices) == N_CORES, f"need {N_CORES} cores, got {len(devices)}"
    mesh = Mesh(np.asarray(devices), ("core",))
    in_specs = (PartitionSpec("core"),) * (n_params + n_outs)
    out_specs = (PartitionSpec("core"),) * n_outs
    sharded = jax.jit(
        shard_map(
            _body, mesh=mesh, in_specs=in_specs, out_specs=out_specs, check_rep=False
        ),
        donate_argnums=tuple(range(n_params, n_params + n_outs)),
        keep_unused=True,
    )
    return sharded, in_names, out_names, out_avals, zero_outs


def _prep_inputs(A: np.ndarray, h: np.ndarray):
    A_bf = _bf16(A)  # [B, N, N] bf16
    hh = np.zeros((16, C), np.float32)
    hh[0:NUM_TERMS] = h[1 : NUM_TERMS + 1]
    hh[NUM_TERMS] = h[0]
    return A_bf, hh


def kernel(A: np.ndarray, h: np.ndarray) -> np.ndarray:
    global _RUNNER, _NC
    A = np.ascontiguousarray(A, np.float32)
    h = np.ascontiguousarray(h, np.float32)
    if _RUNNER is None:
        _NC = _build(N)
        _RUNNER = _make_runner(_NC)
    sharded, in_names, out_names, out_avals, zero_outs = _RUNNER

    A_bf, hh = _prep_inputs(A, h)
    per_name = {
        "A": A_bf.reshape(BATCH * N, N),
        "hh": np.ascontiguousarray(np.broadcast_to(hh, (BATCH, 16, C))).reshape(
            BATCH * 16, C
        ),
    }
    concat_in = [per_name[name] for name in in_names]
    concat_zeros = [
        np.zeros((BATCH * z.shape[0], *z.shape[1:]), z.dtype) for z in zero_outs
    ]
    outs = sharded(*concat_in, *concat_zeros)
    y = np.asarray(outs[out_names.index("Y")]).reshape(BATCH, N, C)
    return np.ascontiguousarray(y, np.float32)
